# revision 2
# baseline (speedup 1.0000x reference)
"""CrossBatchAttention kernel for 8 Trainium2 NeuronCores.

Strategy: tensor-parallel over heads. 16 heads / 8 cores = 2 heads per core.
Each core computes:
  - v     = (hidden @ Wv_slice.T) in [b, e] layout, kt-outer so it overlaps
    the streaming-in of hidden.T (16 accumulators fill all 8 PSUM banks)
  - qT/kT = (W_slice @ hidden.T)  in [e, b] layout (e = local head*128 + d)
  - per (head, q-half) sub-phase: scoresT[k, q] = kT-tiles @ qT (PE-native),
    exp on ACT, diagonal zeroed multiplicatively, row-sums via ones-matmul
    partition reduction, 1/sum broadcast back to [128, q] via a second
    ones-matmul, ctxT[d, q] = v-tiles @ expT accumulated in PSUM, normalized
    on eviction. Epilogue matmuls are software-pipelined into the next
    sub-phase so the PE never stalls on the ACT/DVE normalization chain.
  - cross_partial[b, o] = ctxT-tiles @ WoT_slice (accumulate 2 head tiles)
Host: sums the 8 partial cross projections, out = hidden + sigmoid(s)*cross.

All matmul inputs bf16 (fp32 PSUM accumulation). The residual path keeps
hidden in fp32 exactly, and cross contributes only ~2% of output magnitude,
so bf16 compute error is negligible end-to-end.
"""

import numpy as np
import ml_dtypes

B = 2048
H = 2048
NH = 16
HD = 128
NCORES = 8
HL = NH // NCORES          # heads per core = 2
E = HL * HD                # local projection width = 256
P = 128
KT = H // P                # 16 contraction tiles over hidden dim
BT = B // P                # 16 row tiles

_BF16 = ml_dtypes.bfloat16

_compiled = None


def _build():
    import concourse.bass as bass  # noqa: F401
    import concourse.tile as tile
    from concourse import bacc, mybir

    bf = mybir.dt.bfloat16
    f32 = mybir.dt.float32
    Exp = mybir.ActivationFunctionType.Exp
    mult = mybir.AluOpType.mult

    nc = bacc.Bacc(
        "TRN2",
        target_bir_lowering=False,
        debug=False,
        enable_asserts=False,
        num_devices=NCORES,
    )

    hT_d = nc.dram_tensor("hT", [H, B], bf, kind="ExternalInput").ap()
    wqT_d = nc.dram_tensor("wqT", [H, E], bf, kind="ExternalInput").ap()
    wkT_d = nc.dram_tensor("wkT", [H, E], bf, kind="ExternalInput").ap()
    wvT_d = nc.dram_tensor("wvT", [H, E], bf, kind="ExternalInput").ap()
    woT_d = nc.dram_tensor("woT", [E, H], bf, kind="ExternalInput").ap()
    antiI_d = nc.dram_tensor("antiI", [P, P], bf, kind="ExternalInput").ap()
    out_d = nc.dram_tensor("out", [B, H], bf, kind="ExternalOutput").ap()

    with tile.TileContext(nc) as tc:
        with (
            tc.tile_pool(name="const", bufs=1) as constp,
            tc.tile_pool(name="work", bufs=1) as workp,
            tc.tile_pool(name="stream", bufs=4) as streamp,
            tc.tile_pool(name="psA", bufs=2, space="PSUM") as psA,
            tc.tile_pool(name="psB", bufs=2, space="PSUM") as psB,
        ):
            # ---------------- constants ----------------
            antiI = constp.tile([P, P], bf)
            nc.sync.dma_start(antiI[:], antiI_d[:])
            ones_k = constp.tile([P, 1], bf)
            nc.gpsimd.memset(ones_k[:], 1.0)
            ones_m = constp.tile([1, P], f32)
            nc.gpsimd.memset(ones_m[:], 1.0)
            zbias = constp.tile([P, 1], f32)
            nc.gpsimd.memset(zbias[:], 0.0)

            # ---------------- input DMA, interleaved per kt ----------------
            hT_sb = constp.tile([P, KT * B], bf)
            wq_sb = constp.tile([P, KT * E], bf)
            wk_sb = constp.tile([P, KT * E], bf)
            wv_sb = constp.tile([P, KT * E], bf)
            wo_sb = constp.tile([P, HL * H], bf)
            for kt in range(KT):
                for c2 in range(2):
                    nc.sync.dma_start(
                        hT_sb[:, kt * B + c2 * 1024: kt * B + (c2 + 1) * 1024],
                        hT_d[kt * P:(kt + 1) * P, c2 * 1024:(c2 + 1) * 1024],
                    )
                nc.sync.dma_start(wv_sb[:, kt * E:(kt + 1) * E], wvT_d[kt * P:(kt + 1) * P, :])
                nc.sync.dma_start(wq_sb[:, kt * E:(kt + 1) * E], wqT_d[kt * P:(kt + 1) * P, :])
                nc.sync.dma_start(wk_sb[:, kt * E:(kt + 1) * E], wkT_d[kt * P:(kt + 1) * P, :])
            for et in range(HL):
                nc.sync.dma_start(wo_sb[:, et * H:(et + 1) * H], woT_d[et * P:(et + 1) * P, :])

            qT_sb = workp.tile([P, HL * B], bf)   # [d, b] per head at h*B
            kT_sb = workp.tile([P, HL * B], bf)
            v_sb = workp.tile([P, BT * E], bf)    # [b%128, bt*E + e]
            ctxT_sb = workp.tile([P, HL * B], bf)

            # ------- v projection, kt-outer, overlapped with input DMA -------
            # 16 accumulators [128, 256] packed into 4 psum tiles (all 8 banks)
            vac0 = psA.tile([P, 1024], f32, tag="A")
            vac1 = psA.tile([P, 1024], f32, tag="A")
            vac2 = psB.tile([P, 1024], f32, tag="B")
            vac3 = psB.tile([P, 1024], f32, tag="B")
            vtiles = [vac0, vac1, vac2, vac3]
            for kt in range(KT):
                for bt in range(BT):
                    t = vtiles[bt // 4]
                    sl = (bt % 4) * 256
                    nc.tensor.matmul(
                        t[:, sl:sl + 256],
                        lhsT=hT_sb[:, kt * B + bt * P: kt * B + (bt + 1) * P],
                        rhs=wv_sb[:, kt * E:(kt + 1) * E],
                        start=(kt == 0),
                        stop=(kt == KT - 1),
                    )
            for bt in range(BT):
                t = vtiles[bt // 4]
                sl = (bt % 4) * 256
                nc.any.tensor_copy(v_sb[:, bt * E:(bt + 1) * E], t[:, sl:sl + 256])

            # ---------------- q/k projections (out = [e, b]) ----------------
            for dst, w_sb in ((qT_sb, wq_sb), (kT_sb, wk_sb)):
                for et in range(HL):
                    for bh in range(B // 1024):
                        psp = psA.tile([P, 1024], f32, tag="A")
                        for kt in range(KT):
                            for b2 in range(2):
                                bc = bh * 2 + b2
                                nc.tensor.matmul(
                                    psp[:, b2 * 512:(b2 + 1) * 512],
                                    lhsT=w_sb[:, kt * E + et * P: kt * E + (et + 1) * P],
                                    rhs=hT_sb[:, kt * B + bc * 512: kt * B + (bc + 1) * 512],
                                    start=(kt == 0),
                                    stop=(kt == KT - 1),
                                )
                        nc.any.tensor_copy(dst[:, et * B + bh * 1024: et * B + (bh + 1) * 1024], psp[:])

            # ---------------- attention: 4 (head, q-half) sub-phases ----------
            # epilogue of sub-phase s is emitted inside sub-phase s+1's kt loop
            # (part 1 at kt==3, part 2 at kt==6) so the PE stream never waits
            # on the ACT/DVE normalization chain.
            pending = [None]

            def epilogue_part1(st):
                h, qh, acc = st["h"], st["qh"], st["acc"]
                st["rrows"] = []
                for q2 in range(2):
                    pssum = psA.tile([1, 512], f32, tag="A", name=f"pssum_{h}_{qh}_{q2}")
                    nc.tensor.matmul(
                        pssum[:], lhsT=ones_k[:], rhs=acc[:, q2 * 512:(q2 + 1) * 512],
                        start=True, stop=True,
                    )
                    rrow = workp.tile([1, 512], f32, tag="rrow", bufs=4,
                                      name=f"rrow_{h}_{qh}_{q2}")
                    nc.vector.reciprocal(rrow[:], pssum[:])
                    st["rrows"].append(rrow)

            def epilogue_part2(st):
                h, qh, ctxps = st["h"], st["qh"], st["ctxps"]
                for q2 in range(2):
                    qc = qh * 2 + q2
                    psrb = psA.tile([P, 512], f32, tag="A", name=f"psrb_{h}_{qh}_{q2}")
                    nc.tensor.matmul(psrb[:], lhsT=ones_m[:], rhs=st["rrows"][q2][:],
                                     start=True, stop=True)
                    rb = workp.tile([P, 512], f32, tag="rb", bufs=4,
                                    name=f"rb_{h}_{qh}_{q2}")
                    nc.scalar.copy(rb[:], psrb[:])
                    nc.vector.tensor_tensor(
                        ctxT_sb[:, h * B + qc * 512: h * B + (qc + 1) * 512],
                        ctxps[:, q2 * 512:(q2 + 1) * 512],
                        rb[:],
                        op=mult,
                    )

            def flush_pending(trigger_kt=None):
                st = pending[0]
                if st is None:
                    return
                if trigger_kt is None or trigger_kt == 3:
                    if not st.get("p1"):
                        epilogue_part1(st)
                        st["p1"] = True
                if trigger_kt is None or trigger_kt == 6:
                    if st.get("p1") and not st.get("p2"):
                        epilogue_part2(st)
                        st["p2"] = True
                        pending[0] = None

            for h in range(HL):
                for qh in range(2):
                    ctxps = psB.tile([P, 1024], f32, tag="B", name=f"ctxps_{h}_{qh}")
                    acc = workp.tile([P, 1024], bf, tag="acc", bufs=2, name=f"acc_{h}_{qh}")
                    for kt in range(KT):
                        flush_pending(kt)
                        pss = psA.tile([P, 1024], f32, tag="A", name=f"pss_{h}_{qh}_{kt}")
                        for q2 in range(2):
                            qc = qh * 2 + q2
                            nc.tensor.matmul(
                                pss[:, q2 * 512:(q2 + 1) * 512],
                                lhsT=kT_sb[:, h * B + kt * P: h * B + (kt + 1) * P],
                                rhs=qT_sb[:, h * B + qc * 512: h * B + (qc + 1) * 512],
                                start=True,
                                stop=True,
                            )
                        ex = streamp.tile([P, 1024], bf, tag="exp", name=f"ex_{h}_{qh}_{kt}")
                        nc.scalar.activation(ex[:], pss[:], Exp, bias=zbias[:, 0:1])
                        if kt // 8 == qh:
                            off = kt * P - qh * 1024
                            nc.vector.tensor_tensor(
                                ex[:, off:off + P], ex[:, off:off + P], antiI[:], op=mult
                            )
                        if kt == 0:
                            nc.vector.tensor_copy(acc[:], ex[:])
                        else:
                            nc.vector.tensor_add(acc[:], acc[:], ex[:])
                        for q2 in range(2):
                            nc.tensor.matmul(
                                ctxps[:, q2 * 512:(q2 + 1) * 512],
                                lhsT=v_sb[:, kt * E + h * P: kt * E + h * P + P],
                                rhs=ex[:, q2 * 512:(q2 + 1) * 512],
                                start=(kt == 0),
                                stop=(kt == KT - 1),
                            )
                    flush_pending()  # anything still pending from previous sub-phase
                    pending[0] = {"h": h, "qh": qh, "acc": acc, "ctxps": ctxps}

            # ---------------- output projection (partial) ----------------
            # bt 0..7 depend on sub-phases (h,qh=0); bt 8..15 on (h,qh=1).
            # The last sub-phase's epilogue is pipelined into the first
            # cross iterations (which only need qh=0 columns).
            for bt in range(BT):
                if bt == 0:
                    flush_pending(3)
                if bt == 2:
                    flush_pending(6)
                for oh in range(2):
                    psx = psA.tile([P, 1024], f32, tag="A", name=f"psx_{bt}_{oh}")
                    for et in range(HL):
                        for o2 in range(2):
                            nc.tensor.matmul(
                                psx[:, o2 * 512:(o2 + 1) * 512],
                                lhsT=ctxT_sb[:, et * B + bt * P: et * B + (bt + 1) * P],
                                rhs=wo_sb[:, et * H + oh * 1024 + o2 * 512: et * H + oh * 1024 + (o2 + 1) * 512],
                                start=(et == 0),
                                stop=(et == HL - 1),
                            )
                    xo = streamp.tile([P, 1024], bf, tag="xo", name=f"xo_{bt}_{oh}")
                    nc.any.tensor_copy(xo[:], psx[:])
                    nc.sync.dma_start(
                        out_d[bt * P:(bt + 1) * P, oh * 1024:(oh + 1) * 1024], xo[:]
                    )

    nc.compile()
    return nc


def _get_compiled():
    global _compiled
    if _compiled is None:
        _compiled = _build()
    return _compiled


def _numpy_reference(hidden_states, attention_mask, Wq, Wk, Wv, Wo, scale_param):
    hs = np.asarray(hidden_states, np.float64)
    q = (hs @ np.asarray(Wq, np.float64).T).reshape(B, NH, HD).transpose(1, 0, 2)
    k = (hs @ np.asarray(Wk, np.float64).T).reshape(B, NH, HD).transpose(1, 0, 2)
    v = (hs @ np.asarray(Wv, np.float64).T).reshape(B, NH, HD).transpose(1, 0, 2)
    scores = np.einsum("hqd,hkd->hqk", q, k) / (HD ** 0.5)
    eye = np.eye(B, dtype=bool)
    scores = np.where(eye[None, :, :], -np.inf, scores)
    mask = np.asarray(attention_mask, bool)
    scores = np.where(mask[None, None, :], scores, -np.inf)
    m = scores.max(axis=-1, keepdims=True)
    m = np.where(np.isfinite(m), m, 0.0)
    e = np.exp(scores - m)
    s = e.sum(axis=-1, keepdims=True)
    attn = np.where(s > 0, e / np.maximum(s, 1e-300), 0.0)
    ctx = np.einsum("hqk,hkd->hqd", attn, v)
    ctx = ctx.transpose(1, 0, 2).reshape(B, H)
    cross = ctx @ np.asarray(Wo, np.float64).T
    scale = 1.0 / (1.0 + np.exp(-float(np.asarray(scale_param).reshape(-1)[0])))
    return (hs + scale * cross).astype(np.float32)


LAST_RESULTS = None


def kernel(hidden_states, attention_mask, Wq, Wk, Wv, Wo, scale_param):
    hs = np.asarray(hidden_states, np.float32)
    mask = np.asarray(attention_mask, bool)
    if not mask.all():
        return _numpy_reference(hidden_states, mask, Wq, Wk, Wv, Wo, scale_param)

    from concourse import bass_utils

    nc = _get_compiled()

    hT = np.ascontiguousarray(hs.T).astype(_BF16)
    antiI = (1.0 - np.eye(P, dtype=np.float32)).astype(_BF16)
    Wq = np.asarray(Wq, np.float32)
    Wk = np.asarray(Wk, np.float32)
    Wv = np.asarray(Wv, np.float32)
    Wo = np.asarray(Wo, np.float32)

    in_maps = []
    for c in range(NCORES):
        rs = slice(c * E, (c + 1) * E)
        in_maps.append({
            "hT": hT,
            "wqT": np.ascontiguousarray(Wq[rs, :].T / np.float32(HD ** 0.5)).astype(_BF16),
            "wkT": np.ascontiguousarray(Wk[rs, :].T).astype(_BF16),
            "wvT": np.ascontiguousarray(Wv[rs, :].T).astype(_BF16),
            "woT": np.ascontiguousarray(Wo[:, rs].T).astype(_BF16),
            "antiI": antiI,
        })

    import os
    res = bass_utils.run_bass_kernel_spmd(
        nc, in_maps, core_ids=list(range(NCORES)),
        trace=bool(os.environ.get("KERNEL_TRACE")),
    )
    global LAST_RESULTS
    LAST_RESULTS = res

    cross = np.zeros((B, H), np.float32)
    for r in res.results:
        cross += np.asarray(r["out"], np.float32)
    scale = np.float32(1.0 / (1.0 + np.exp(-float(np.asarray(scale_param).reshape(-1)[0]))))
    return (hs + scale * cross).astype(np.float32)


# revision 4
# speedup vs baseline: 1.0037x; 1.0037x over previous
"""CrossBatchAttention kernel for 8 Trainium2 NeuronCores.

Strategy: tensor-parallel over heads. 16 heads / 8 cores = 2 heads per core.
Each core computes:
  - v     = (hidden @ Wv_slice.T) in [b, e] layout, kt-outer in two waves of
    8 bank-aligned PSUM accumulators so it overlaps the streaming-in of
    hidden.T
  - qT/kT = (W_slice @ hidden.T)  in [e, b] layout (e = local head*128 + d)
  - per (head, q-half) sub-phase: scoresT[k, q] = kT-tiles @ qT (PE-native),
    exp on ACT, diagonal zeroed multiplicatively, row-sums via ones-matmul
    partition reduction, 1/sum broadcast back to [128, q] via a second
    ones-matmul, ctxT[d, q] = v-tiles @ expT accumulated in PSUM, normalized
    on eviction. Epilogue matmuls are software-pipelined into the next
    sub-phase, and the first output-projection groups are interleaved into
    the last sub-phase, so the PE never stalls on the ACT/DVE chain.
  - cross_partial[b, o] = ctxT-tiles @ WoT_slice (accumulate 2 head tiles)
Host: sums the 8 partial cross projections, out = hidden + sigmoid(s)*cross.

All matmul inputs bf16 (fp32 PSUM accumulation). The residual path keeps
hidden in fp32 exactly, and cross contributes only ~2% of output magnitude,
so bf16 compute error is negligible end-to-end.
"""

import numpy as np
import ml_dtypes

B = 2048
H = 2048
NH = 16
HD = 128
NCORES = 8
HL = NH // NCORES          # heads per core = 2
E = HL * HD                # local projection width = 256
P = 128
KT = H // P                # 16 contraction tiles over hidden dim
BT = B // P                # 16 row tiles

_BF16 = ml_dtypes.bfloat16

_compiled = None


def _build():
    import concourse.bass as bass  # noqa: F401
    import concourse.tile as tile
    from concourse import bacc, mybir

    bf = mybir.dt.bfloat16
    f32 = mybir.dt.float32
    Exp = mybir.ActivationFunctionType.Exp
    mult = mybir.AluOpType.mult

    nc = bacc.Bacc(
        "TRN2",
        target_bir_lowering=False,
        debug=False,
        enable_asserts=False,
        num_devices=NCORES,
    )

    hT_d = nc.dram_tensor("hT", [H, B], bf, kind="ExternalInput").ap()
    wqT_d = nc.dram_tensor("wqT", [H, E], bf, kind="ExternalInput").ap()
    wkT_d = nc.dram_tensor("wkT", [H, E], bf, kind="ExternalInput").ap()
    wvT_d = nc.dram_tensor("wvT", [H, E], bf, kind="ExternalInput").ap()
    woT_d = nc.dram_tensor("woT", [E, H], bf, kind="ExternalInput").ap()
    antiI_d = nc.dram_tensor("antiI", [P, P], bf, kind="ExternalInput").ap()
    out_d = nc.dram_tensor("out", [B, H], bf, kind="ExternalOutput").ap()

    with tile.TileContext(nc) as tc:
        with (
            tc.tile_pool(name="const", bufs=1) as constp,
            tc.tile_pool(name="work", bufs=1) as workp,
            tc.tile_pool(name="stream", bufs=6) as streamp,
            tc.tile_pool(name="psA", bufs=2, space="PSUM") as psA,
            tc.tile_pool(name="psB", bufs=2, space="PSUM") as psB,
        ):
            # ---------------- constants ----------------
            antiI = constp.tile([P, P], bf)
            nc.sync.dma_start(antiI[:], antiI_d[:])
            ones_k = constp.tile([P, 1], bf)
            nc.gpsimd.memset(ones_k[:], 1.0)
            ones_m = constp.tile([1, P], f32)
            nc.gpsimd.memset(ones_m[:], 1.0)
            zbias = constp.tile([P, 1], f32)
            nc.gpsimd.memset(zbias[:], 0.0)

            # ------- input DMA, ordered to match consumption order -------
            hT_sb = constp.tile([P, KT * B], bf)
            wq_sb = constp.tile([P, KT * E], bf)
            wk_sb = constp.tile([P, KT * E], bf)
            wv_sb = constp.tile([P, KT * E], bf)
            wo_sb = constp.tile([P, HL * H], bf)

            def dma_h(kt, c2):
                nc.sync.dma_start(
                    hT_sb[:, kt * B + c2 * 1024: kt * B + (c2 + 1) * 1024],
                    hT_d[kt * P:(kt + 1) * P, c2 * 1024:(c2 + 1) * 1024],
                )

            def dma_w(w_sb, w_d, kt):
                nc.sync.dma_start(w_sb[:, kt * E:(kt + 1) * E], w_d[kt * P:(kt + 1) * P, :])

            for kt in range(8):
                dma_h(kt, 0)
                dma_h(kt, 1)
                dma_w(wv_sb, wvT_d, kt)
            for kt in range(8, KT):
                dma_h(kt, 0)
                dma_h(kt, 1)
                dma_w(wv_sb, wvT_d, kt)
                dma_w(wq_sb, wqT_d, 2 * (kt - 8))
                dma_w(wq_sb, wqT_d, 2 * (kt - 8) + 1)
            for kt in range(KT):
                dma_w(wk_sb, wkT_d, kt)
            for et in range(HL):
                nc.sync.dma_start(wo_sb[:, et * H:(et + 1) * H], woT_d[et * P:(et + 1) * P, :])

            qT_sb = workp.tile([P, HL * B], bf)   # [d, b] per head at h*B
            kT_sb = workp.tile([P, HL * B], bf)
            v_sb = workp.tile([P, BT * E], bf)    # [b%128, bt*E + e]
            ctxT_sb = workp.tile([P, HL * B], bf)

            # ------- v projection, kt-outer, 2 waves of 8 bank-aligned accs ----
            for wave in range(2):
                vt0 = psA.tile([P, 1024], f32, tag="A", name=f"vt0_{wave}")
                vt1 = psA.tile([P, 1024], f32, tag="A", name=f"vt1_{wave}")
                vt2 = psB.tile([P, 1024], f32, tag="B", name=f"vt2_{wave}")
                vt3 = psB.tile([P, 1024], f32, tag="B", name=f"vt3_{wave}")
                vtiles = [vt0, vt1, vt2, vt3]

                def vacc(j):
                    # accumulator j -> own PSUM bank: slot j//2, col (j%2)*512
                    return vtiles[j // 2][:, (j % 2) * 512: (j % 2) * 512 + 256]

                for kt in range(KT):
                    for j in range(8):
                        bt = wave * 8 + j
                        nc.tensor.matmul(
                            vacc(j),
                            lhsT=hT_sb[:, kt * B + bt * P: kt * B + (bt + 1) * P],
                            rhs=wv_sb[:, kt * E:(kt + 1) * E],
                            start=(kt == 0),
                            stop=(kt == KT - 1),
                        )
                for j in range(8):
                    bt = wave * 8 + j
                    nc.any.tensor_copy(v_sb[:, bt * E:(bt + 1) * E], vacc(j))

            # ---------------- q/k projections (out = [e, b]) ----------------
            for dst, w_sb in ((qT_sb, wq_sb), (kT_sb, wk_sb)):
                for et in range(HL):
                    for bh in range(B // 1024):
                        psp = psA.tile([P, 1024], f32, tag="A", name=f"psp_{et}_{bh}")
                        for kt in range(KT):
                            for b2 in range(2):
                                bc = bh * 2 + b2
                                nc.tensor.matmul(
                                    psp[:, b2 * 512:(b2 + 1) * 512],
                                    lhsT=w_sb[:, kt * E + et * P: kt * E + (et + 1) * P],
                                    rhs=hT_sb[:, kt * B + bc * 512: kt * B + (bc + 1) * 512],
                                    start=(kt == 0),
                                    stop=(kt == KT - 1),
                                )
                        nc.any.tensor_copy(dst[:, et * B + bh * 1024: et * B + (bh + 1) * 1024], psp[:])

            # ------------- output projection helper (interleavable) -----------
            def emit_cross(bt, oh, idx):
                psx = psA.tile([P, 1024], f32, tag="A", name=f"psx_{bt}_{oh}")
                for et in range(HL):
                    for o2 in range(2):
                        nc.tensor.matmul(
                            psx[:, o2 * 512:(o2 + 1) * 512],
                            lhsT=ctxT_sb[:, et * B + bt * P: et * B + (bt + 1) * P],
                            rhs=wo_sb[:, et * H + oh * 1024 + o2 * 512: et * H + oh * 1024 + (o2 + 1) * 512],
                            start=(et == 0),
                            stop=(et == HL - 1),
                        )
                xo = streamp.tile([P, 1024], bf, tag="xo", name=f"xo_{bt}_{oh}")
                if idx % 2 == 0:
                    nc.vector.tensor_copy(xo[:], psx[:])
                else:
                    nc.scalar.copy(xo[:], psx[:])
                nc.sync.dma_start(
                    out_d[bt * P:(bt + 1) * P, oh * 1024:(oh + 1) * 1024], xo[:]
                )

            # ---------------- attention: 4 (head, q-half) sub-phases ----------
            # epilogue of sub-phase s is software-pipelined into sub-phase s+1
            # (part 1 at kt==4, part 2 at kt==8); the first cross-projection
            # groups are interleaved into the last sub-phase's kt loop.
            pending = [None]

            def epilogue_part1():
                st = pending[0]
                if st is None or st.get("p1"):
                    return
                st["p1"] = True
                h, qh, acc = st["h"], st["qh"], st["acc"]
                st["rrows"] = []
                for q2 in range(2):
                    pssum = psA.tile([1, 512], f32, tag="A", name=f"pssum_{h}_{qh}_{q2}")
                    nc.tensor.matmul(
                        pssum[:], lhsT=ones_k[:], rhs=acc[:, q2 * 512:(q2 + 1) * 512],
                        start=True, stop=True,
                    )
                    rrow = workp.tile([1, 512], f32, tag="rrow", bufs=4,
                                      name=f"rrow_{h}_{qh}_{q2}")
                    nc.vector.reciprocal(rrow[:], pssum[:])
                    st["rrows"].append(rrow)

            def epilogue_part2():
                st = pending[0]
                if st is None or not st.get("p1") or st.get("p2"):
                    return
                st["p2"] = True
                h, qh, ctxps = st["h"], st["qh"], st["ctxps"]
                for q2 in range(2):
                    qc = qh * 2 + q2
                    psrb = psA.tile([P, 512], f32, tag="A", name=f"psrb_{h}_{qh}_{q2}")
                    nc.tensor.matmul(psrb[:], lhsT=ones_m[:], rhs=st["rrows"][q2][:],
                                     start=True, stop=True)
                    rb = workp.tile([P, 512], f32, tag="rb", bufs=4,
                                    name=f"rb_{h}_{qh}_{q2}")
                    nc.vector.tensor_copy(rb[:], psrb[:])
                    nc.vector.tensor_tensor(
                        ctxT_sb[:, h * B + qc * 512: h * B + (qc + 1) * 512],
                        ctxps[:, q2 * 512:(q2 + 1) * 512],
                        rb[:],
                        op=mult,
                    )
                pending[0] = None

            # cross groups interleaved into the last sub-phase (need only
            # qh=0 columns, i.e. sub-phases 0 and 2)
            early_cross = [(0, 0), (0, 1), (1, 0), (1, 1), (2, 0), (2, 1)]

            for h in range(HL):
                for qh in range(2):
                    last = (h == HL - 1 and qh == 1)
                    ctxps = psB.tile([P, 1024], f32, tag="B", name=f"ctxps_{h}_{qh}")
                    acc = workp.tile([P, 1024], bf, tag="acc", bufs=2, name=f"acc_{h}_{qh}")
                    for kt in range(KT):
                        if kt == 4:
                            epilogue_part1()
                        if kt == 8:
                            epilogue_part2()
                        if last and kt >= 10:
                            bt, oh = early_cross[kt - 10]
                            emit_cross(bt, oh, kt)
                        pss = psA.tile([P, 1024], f32, tag="A", name=f"pss_{h}_{qh}_{kt}")
                        for q2 in range(2):
                            qc = qh * 2 + q2
                            nc.tensor.matmul(
                                pss[:, q2 * 512:(q2 + 1) * 512],
                                lhsT=kT_sb[:, h * B + kt * P: h * B + (kt + 1) * P],
                                rhs=qT_sb[:, h * B + qc * 512: h * B + (qc + 1) * 512],
                                start=True,
                                stop=True,
                            )
                        ex = streamp.tile([P, 1024], bf, tag="exp", name=f"ex_{h}_{qh}_{kt}")
                        nc.scalar.activation(ex[:], pss[:], Exp, bias=zbias[:, 0:1])
                        if kt // 8 == qh:
                            off = kt * P - qh * 1024
                            nc.vector.tensor_tensor(
                                ex[:, off:off + P], ex[:, off:off + P], antiI[:], op=mult
                            )
                        if kt == 0:
                            nc.vector.tensor_copy(acc[:], ex[:])
                        else:
                            nc.vector.tensor_add(acc[:], acc[:], ex[:])
                        for q2 in range(2):
                            nc.tensor.matmul(
                                ctxps[:, q2 * 512:(q2 + 1) * 512],
                                lhsT=v_sb[:, kt * E + h * P: kt * E + h * P + P],
                                rhs=ex[:, q2 * 512:(q2 + 1) * 512],
                                start=(kt == 0),
                                stop=(kt == KT - 1),
                            )
                    epilogue_part1()
                    epilogue_part2()
                    pending[0] = {"h": h, "qh": qh, "acc": acc, "ctxps": ctxps}

            # ---------------- remaining output projection ----------------
            idx = len(early_cross)
            done = set(early_cross)
            for bt in range(BT):
                if bt == 3:
                    epilogue_part1()
                if bt == 5:
                    epilogue_part2()
                for oh in range(2):
                    if (bt, oh) in done:
                        continue
                    emit_cross(bt, oh, idx)
                    idx += 1

    nc.compile()
    return nc


def _get_compiled():
    global _compiled
    if _compiled is None:
        _compiled = _build()
    return _compiled


def _numpy_reference(hidden_states, attention_mask, Wq, Wk, Wv, Wo, scale_param):
    hs = np.asarray(hidden_states, np.float64)
    q = (hs @ np.asarray(Wq, np.float64).T).reshape(B, NH, HD).transpose(1, 0, 2)
    k = (hs @ np.asarray(Wk, np.float64).T).reshape(B, NH, HD).transpose(1, 0, 2)
    v = (hs @ np.asarray(Wv, np.float64).T).reshape(B, NH, HD).transpose(1, 0, 2)
    scores = np.einsum("hqd,hkd->hqk", q, k) / (HD ** 0.5)
    eye = np.eye(B, dtype=bool)
    scores = np.where(eye[None, :, :], -np.inf, scores)
    mask = np.asarray(attention_mask, bool)
    scores = np.where(mask[None, None, :], scores, -np.inf)
    m = scores.max(axis=-1, keepdims=True)
    m = np.where(np.isfinite(m), m, 0.0)
    e = np.exp(scores - m)
    s = e.sum(axis=-1, keepdims=True)
    attn = np.where(s > 0, e / np.maximum(s, 1e-300), 0.0)
    ctx = np.einsum("hqk,hkd->hqd", attn, v)
    ctx = ctx.transpose(1, 0, 2).reshape(B, H)
    cross = ctx @ np.asarray(Wo, np.float64).T
    scale = 1.0 / (1.0 + np.exp(-float(np.asarray(scale_param).reshape(-1)[0])))
    return (hs + scale * cross).astype(np.float32)


LAST_RESULTS = None


def kernel(hidden_states, attention_mask, Wq, Wk, Wv, Wo, scale_param):
    hs = np.asarray(hidden_states, np.float32)
    mask = np.asarray(attention_mask, bool)
    if not mask.all():
        return _numpy_reference(hidden_states, mask, Wq, Wk, Wv, Wo, scale_param)

    from concourse import bass_utils

    nc = _get_compiled()

    hT = np.ascontiguousarray(hs.T).astype(_BF16)
    antiI = (1.0 - np.eye(P, dtype=np.float32)).astype(_BF16)
    Wq = np.asarray(Wq, np.float32)
    Wk = np.asarray(Wk, np.float32)
    Wv = np.asarray(Wv, np.float32)
    Wo = np.asarray(Wo, np.float32)

    in_maps = []
    for c in range(NCORES):
        rs = slice(c * E, (c + 1) * E)
        in_maps.append({
            "hT": hT,
            "wqT": np.ascontiguousarray(Wq[rs, :].T / np.float32(HD ** 0.5)).astype(_BF16),
            "wkT": np.ascontiguousarray(Wk[rs, :].T).astype(_BF16),
            "wvT": np.ascontiguousarray(Wv[rs, :].T).astype(_BF16),
            "woT": np.ascontiguousarray(Wo[:, rs].T).astype(_BF16),
            "antiI": antiI,
        })

    import os
    res = bass_utils.run_bass_kernel_spmd(
        nc, in_maps, core_ids=list(range(NCORES)),
        trace=bool(os.environ.get("KERNEL_TRACE")),
    )
    global LAST_RESULTS
    LAST_RESULTS = res

    cross = np.zeros((B, H), np.float32)
    for r in res.results:
        cross += np.asarray(r["out"], np.float32)
    scale = np.float32(1.0 / (1.0 + np.exp(-float(np.asarray(scale_param).reshape(-1)[0]))))
    return (hs + scale * cross).astype(np.float32)


# revision 8
# speedup vs baseline: 1.1531x; 1.1489x over previous
"""CrossBatchAttention kernel for 8 Trainium2 NeuronCores.

Strategy: tensor-parallel over heads. 16 heads / 8 cores = 2 heads per core.
Each core computes:
  - v     = (hidden @ Wv_slice.T) in [b, e] layout, kt-outer in two waves of
    8 bank-aligned PSUM accumulators so it overlaps the streaming-in of
    hidden.T
  - qT/kT = (W_slice @ hidden.T)  in [e, b] layout (e = local head*128 + d)
  - per (head, q-half) sub-phase: scoresT[k, q] = kT-tiles @ qT (PE-native),
    exp on ACT, diagonal zeroed multiplicatively, row-sums via ones-matmul
    partition reduction, 1/sum broadcast back to [128, q] via a second
    ones-matmul, ctxT[d, q] = v-tiles @ expT accumulated in PSUM, normalized
    on eviction. Epilogue matmuls are software-pipelined into the next
    sub-phase, and the first output-projection groups are interleaved into
    the last sub-phase, so the PE never stalls on the ACT/DVE chain.
  - cross_partial[b, o] = ctxT-tiles @ WoT_slice (accumulate 2 head tiles)
Host: sums the 8 partial cross projections, out = hidden + sigmoid(s)*cross.

All matmul inputs bf16 (fp32 PSUM accumulation). The residual path keeps
hidden in fp32 exactly, and cross contributes only ~2% of output magnitude,
so bf16 compute error is negligible end-to-end.
"""

import numpy as np
import ml_dtypes

B = 2048
H = 2048
NH = 16
HD = 128
NCORES = 8
HL = NH // NCORES          # heads per core = 2
E = HL * HD                # local projection width = 256
P = 128
KT = H // P                # 16 contraction tiles over hidden dim
BT = B // P                # 16 row tiles

_BF16 = ml_dtypes.bfloat16

_compiled = None


def _build():
    import concourse.bass as bass  # noqa: F401
    import concourse.tile as tile
    from concourse import bacc, mybir

    bf = mybir.dt.bfloat16
    f32 = mybir.dt.float32
    Exp = mybir.ActivationFunctionType.Exp
    mult = mybir.AluOpType.mult

    nc = bacc.Bacc(
        "TRN2",
        target_bir_lowering=False,
        debug=False,
        enable_asserts=False,
        num_devices=NCORES,
    )

    # inputs are pre-packed on the host to match SBUF layout exactly, so every
    # DMA row is a large contiguous span (descriptor-efficient)
    hT_d = nc.dram_tensor("hT", [P, KT * B], bf, kind="ExternalInput").ap()
    wqT_d = nc.dram_tensor("wqT", [P, KT * E], bf, kind="ExternalInput").ap()
    wkT_d = nc.dram_tensor("wkT", [P, KT * E], bf, kind="ExternalInput").ap()
    wvT_d = nc.dram_tensor("wvT", [P, KT * E], bf, kind="ExternalInput").ap()
    woT_d = nc.dram_tensor("woT", [P, HL * H], bf, kind="ExternalInput").ap()
    antiI_d = nc.dram_tensor("antiI", [P, P], bf, kind="ExternalInput").ap()
    out_d = nc.dram_tensor("out", [B, H], bf, kind="ExternalOutput").ap()

    with tile.TileContext(nc) as tc:
        with (
            tc.tile_pool(name="const", bufs=1) as constp,
            tc.tile_pool(name="work", bufs=1) as workp,
            tc.tile_pool(name="stream", bufs=6) as streamp,
            tc.tile_pool(name="psA", bufs=2, space="PSUM") as psA,
            tc.tile_pool(name="psB", bufs=2, space="PSUM") as psB,
        ):
            # ---------------- constants ----------------
            antiI = constp.tile([P, P], bf)
            nc.sync.dma_start(antiI[:], antiI_d[:])
            ones_k = constp.tile([P, 1], bf)
            nc.gpsimd.memset(ones_k[:], 1.0)
            ones_m = constp.tile([1, P], f32)
            nc.gpsimd.memset(ones_m[:], 1.0)
            zbias = constp.tile([P, 1], f32)
            nc.gpsimd.memset(zbias[:], 0.0)

            # ------- input DMA, ordered to match consumption order -------
            hT_sb = constp.tile([P, KT * B], bf)
            wq_sb = constp.tile([P, KT * E], bf)
            wk_sb = constp.tile([P, KT * E], bf)
            wv_sb = constp.tile([P, KT * E], bf)
            wo_sb = constp.tile([P, HL * H], bf)

            nc.sync.dma_start(wv_sb[:], wvT_d[:])
            for kt in range(KT):
                for c2 in range(2):
                    nc.sync.dma_start(
                        hT_sb[:, kt * B + c2 * 1024: kt * B + (c2 + 1) * 1024],
                        hT_d[:, kt * B + c2 * 1024: kt * B + (c2 + 1) * 1024],
                    )
            nc.sync.dma_start(wq_sb[:], wqT_d[:])
            nc.sync.dma_start(wk_sb[:], wkT_d[:])
            nc.sync.dma_start(wo_sb[:], woT_d[:])

            qT_sb = workp.tile([P, HL * B], bf)   # [d, b] per head at h*B
            kT_sb = workp.tile([P, HL * B], bf)
            v_sb = workp.tile([P, BT * E], bf)    # [b%128, bt*E + e]
            ctxT_sb = workp.tile([P, HL * B], bf)

            # ------- v projection, kt-outer, 2 waves of 8 bank-aligned accs ----
            for wave in range(2):
                vt0 = psA.tile([P, 1024], f32, tag="A", name=f"vt0_{wave}")
                vt1 = psA.tile([P, 1024], f32, tag="A", name=f"vt1_{wave}")
                vt2 = psB.tile([P, 1024], f32, tag="B", name=f"vt2_{wave}")
                vt3 = psB.tile([P, 1024], f32, tag="B", name=f"vt3_{wave}")
                vtiles = [vt0, vt1, vt2, vt3]

                def vacc(j):
                    # accumulator j -> own PSUM bank: slot j//2, col (j%2)*512
                    return vtiles[j // 2][:, (j % 2) * 512: (j % 2) * 512 + 256]

                for kt in range(KT):
                    for j in range(8):
                        bt = wave * 8 + j
                        nc.tensor.matmul(
                            vacc(j),
                            lhsT=hT_sb[:, kt * B + bt * P: kt * B + (bt + 1) * P],
                            rhs=wv_sb[:, kt * E:(kt + 1) * E],
                            start=(kt == 0),
                            stop=(kt == KT - 1),
                        )
                for j in range(8):
                    bt = wave * 8 + j
                    nc.any.tensor_copy(v_sb[:, bt * E:(bt + 1) * E], vacc(j))

            # ---------------- q/k projections (out = [e, b]) ----------------
            for dst, w_sb in ((qT_sb, wq_sb), (kT_sb, wk_sb)):
                for et in range(HL):
                    for bh in range(B // 1024):
                        psp = psA.tile([P, 1024], f32, tag="A", name=f"psp_{et}_{bh}")
                        for kt in range(KT):
                            for b2 in range(2):
                                bc = bh * 2 + b2
                                nc.tensor.matmul(
                                    psp[:, b2 * 512:(b2 + 1) * 512],
                                    lhsT=w_sb[:, kt * E + et * P: kt * E + (et + 1) * P],
                                    rhs=hT_sb[:, kt * B + bc * 512: kt * B + (bc + 1) * 512],
                                    start=(kt == 0),
                                    stop=(kt == KT - 1),
                                )
                        nc.any.tensor_copy(dst[:, et * B + bh * 1024: et * B + (bh + 1) * 1024], psp[:])

            # ------------- output projection helper (interleavable) -----------
            def emit_cross(bt, oh, idx):
                psx = psA.tile([P, 1024], f32, tag="A", name=f"psx_{bt}_{oh}")
                for et in range(HL):
                    for o2 in range(2):
                        nc.tensor.matmul(
                            psx[:, o2 * 512:(o2 + 1) * 512],
                            lhsT=ctxT_sb[:, et * B + bt * P: et * B + (bt + 1) * P],
                            rhs=wo_sb[:, et * H + oh * 1024 + o2 * 512: et * H + oh * 1024 + (o2 + 1) * 512],
                            start=(et == 0),
                            stop=(et == HL - 1),
                        )
                xo = streamp.tile([P, 1024], bf, tag="xo", name=f"xo_{bt}_{oh}")
                if idx % 2 == 0:
                    nc.vector.tensor_copy(xo[:], psx[:])
                else:
                    nc.scalar.copy(xo[:], psx[:])
                nc.sync.dma_start(
                    out_d[bt * P:(bt + 1) * P, oh * 1024:(oh + 1) * 1024], xo[:]
                )

            # ---------------- attention: 4 (head, q-half) sub-phases ----------
            # epilogue of sub-phase s is software-pipelined into sub-phase s+1
            # (part 1 at kt==4, part 2 at kt==8); the first cross-projection
            # groups are interleaved into the last sub-phase's kt loop.
            pending = [None]

            def epilogue_part1():
                st = pending[0]
                if st is None or st.get("p1"):
                    return
                st["p1"] = True
                h, qh, acc = st["h"], st["qh"], st["acc"]
                st["rrows"] = []
                for q2 in range(2):
                    pssum = psA.tile([1, 512], f32, tag="A", name=f"pssum_{h}_{qh}_{q2}")
                    nc.tensor.matmul(
                        pssum[:], lhsT=ones_k[:], rhs=acc[:, q2 * 512:(q2 + 1) * 512],
                        start=True, stop=True,
                    )
                    rrow = workp.tile([1, 512], f32, tag="rrow", bufs=4,
                                      name=f"rrow_{h}_{qh}_{q2}")
                    # sums are O(2500) and well-conditioned; the ~18-bit
                    # approximation is far more accurate than needed and 5x
                    # faster than the multi-pass exact reciprocal, which
                    # would block the in-order DVE stream for ~3.3us
                    nc.vector.reciprocal_approx_fast(rrow[:], pssum[:])
                    st["rrows"].append(rrow)

            def epilogue_part2():
                st = pending[0]
                if st is None or not st.get("p1") or st.get("p2"):
                    return
                st["p2"] = True
                h, qh, ctxps = st["h"], st["qh"], st["ctxps"]
                for q2 in range(2):
                    qc = qh * 2 + q2
                    psrb = psA.tile([P, 512], f32, tag="A", name=f"psrb_{h}_{qh}_{q2}")
                    nc.tensor.matmul(psrb[:], lhsT=ones_m[:], rhs=st["rrows"][q2][:],
                                     start=True, stop=True)
                    rb = workp.tile([P, 512], f32, tag="rb", bufs=4,
                                    name=f"rb_{h}_{qh}_{q2}")
                    nc.vector.tensor_copy(rb[:], psrb[:])
                    nc.vector.tensor_tensor(
                        ctxT_sb[:, h * B + qc * 512: h * B + (qc + 1) * 512],
                        ctxps[:, q2 * 512:(q2 + 1) * 512],
                        rb[:],
                        op=mult,
                    )
                pending[0] = None

            # cross groups interleaved into the last sub-phase (need only
            # qh=0 columns, i.e. sub-phases 0 and 2)
            early_cross = [(0, 0), (0, 1), (1, 0), (1, 1), (2, 0), (2, 1)]

            for h in range(HL):
                for qh in range(2):
                    last = (h == HL - 1 and qh == 1)
                    ctxps = psB.tile([P, 1024], f32, tag="B", name=f"ctxps_{h}_{qh}")
                    acc = workp.tile([P, 1024], bf, tag="acc", bufs=2, name=f"acc_{h}_{qh}")
                    for kt in range(KT):
                        if kt == 4:
                            epilogue_part1()
                        if kt == 8:
                            epilogue_part2()
                        if last and kt >= 10:
                            bt, oh = early_cross[kt - 10]
                            emit_cross(bt, oh, kt)
                        pss = psA.tile([P, 1024], f32, tag="A", name=f"pss_{h}_{qh}_{kt}")
                        for q2 in range(2):
                            qc = qh * 2 + q2
                            nc.tensor.matmul(
                                pss[:, q2 * 512:(q2 + 1) * 512],
                                lhsT=kT_sb[:, h * B + kt * P: h * B + (kt + 1) * P],
                                rhs=qT_sb[:, h * B + qc * 512: h * B + (qc + 1) * 512],
                                start=True,
                                stop=True,
                            )
                        ex = streamp.tile([P, 1024], bf, tag="exp", name=f"ex_{h}_{qh}_{kt}")
                        nc.scalar.activation(ex[:], pss[:], Exp, bias=zbias[:, 0:1])
                        if kt // 8 == qh:
                            off = kt * P - qh * 1024
                            nc.vector.tensor_tensor(
                                ex[:, off:off + P], ex[:, off:off + P], antiI[:], op=mult
                            )
                        if kt == 0:
                            nc.vector.tensor_copy(acc[:], ex[:])
                        else:
                            nc.vector.tensor_add(acc[:], acc[:], ex[:])
                        for q2 in range(2):
                            nc.tensor.matmul(
                                ctxps[:, q2 * 512:(q2 + 1) * 512],
                                lhsT=v_sb[:, kt * E + h * P: kt * E + h * P + P],
                                rhs=ex[:, q2 * 512:(q2 + 1) * 512],
                                start=(kt == 0),
                                stop=(kt == KT - 1),
                            )
                    epilogue_part1()
                    epilogue_part2()
                    pending[0] = {"h": h, "qh": qh, "acc": acc, "ctxps": ctxps}

            # ---------------- remaining output projection ----------------
            idx = len(early_cross)
            done = set(early_cross)
            for bt in range(BT):
                if bt == 3:
                    epilogue_part1()
                if bt == 5:
                    epilogue_part2()
                for oh in range(2):
                    if (bt, oh) in done:
                        continue
                    emit_cross(bt, oh, idx)
                    idx += 1

    nc.compile()
    return nc


def _get_compiled():
    global _compiled
    if _compiled is None:
        _compiled = _build()
    return _compiled


def _numpy_reference(hidden_states, attention_mask, Wq, Wk, Wv, Wo, scale_param):
    hs = np.asarray(hidden_states, np.float64)
    q = (hs @ np.asarray(Wq, np.float64).T).reshape(B, NH, HD).transpose(1, 0, 2)
    k = (hs @ np.asarray(Wk, np.float64).T).reshape(B, NH, HD).transpose(1, 0, 2)
    v = (hs @ np.asarray(Wv, np.float64).T).reshape(B, NH, HD).transpose(1, 0, 2)
    scores = np.einsum("hqd,hkd->hqk", q, k) / (HD ** 0.5)
    eye = np.eye(B, dtype=bool)
    scores = np.where(eye[None, :, :], -np.inf, scores)
    mask = np.asarray(attention_mask, bool)
    scores = np.where(mask[None, None, :], scores, -np.inf)
    m = scores.max(axis=-1, keepdims=True)
    m = np.where(np.isfinite(m), m, 0.0)
    e = np.exp(scores - m)
    s = e.sum(axis=-1, keepdims=True)
    attn = np.where(s > 0, e / np.maximum(s, 1e-300), 0.0)
    ctx = np.einsum("hqk,hkd->hqd", attn, v)
    ctx = ctx.transpose(1, 0, 2).reshape(B, H)
    cross = ctx @ np.asarray(Wo, np.float64).T
    scale = 1.0 / (1.0 + np.exp(-float(np.asarray(scale_param).reshape(-1)[0])))
    return (hs + scale * cross).astype(np.float32)


LAST_RESULTS = None


def kernel(hidden_states, attention_mask, Wq, Wk, Wv, Wo, scale_param):
    hs = np.asarray(hidden_states, np.float32)
    mask = np.asarray(attention_mask, bool)
    if not mask.all():
        return _numpy_reference(hidden_states, mask, Wq, Wk, Wv, Wo, scale_param)

    from concourse import bass_utils

    nc = _get_compiled()

    def pack(a):
        # [T*128, W] -> [128, T*W]: row p holds tile-t's row p for every t,
        # matching the SBUF destination layout exactly (contiguous DMA rows)
        t = a.shape[0] // P
        return np.ascontiguousarray(
            a.reshape(t, P, a.shape[1]).transpose(1, 0, 2).reshape(P, t * a.shape[1])
        )

    hT = pack(hs.T.astype(_BF16))
    antiI = (1.0 - np.eye(P, dtype=np.float32)).astype(_BF16)
    Wq = np.asarray(Wq, np.float32)
    Wk = np.asarray(Wk, np.float32)
    Wv = np.asarray(Wv, np.float32)
    Wo = np.asarray(Wo, np.float32)

    in_maps = []
    for c in range(NCORES):
        rs = slice(c * E, (c + 1) * E)
        in_maps.append({
            "hT": hT,
            "wqT": pack((Wq[rs, :].T / np.float32(HD ** 0.5)).astype(_BF16)),
            "wkT": pack(Wk[rs, :].T.astype(_BF16)),
            "wvT": pack(Wv[rs, :].T.astype(_BF16)),
            "woT": pack(Wo[:, rs].T.astype(_BF16)),
            "antiI": antiI,
        })

    import os
    res = bass_utils.run_bass_kernel_spmd(
        nc, in_maps, core_ids=list(range(NCORES)),
        trace=bool(os.environ.get("KERNEL_TRACE")),
    )
    global LAST_RESULTS
    LAST_RESULTS = res

    cross = np.zeros((B, H), np.float32)
    for r in res.results:
        cross += np.asarray(r["out"], np.float32)
    scale = np.float32(1.0 / (1.0 + np.exp(-float(np.asarray(scale_param).reshape(-1)[0]))))
    return (hs + scale * cross).astype(np.float32)


# revision 9
# speedup vs baseline: 1.1892x; 1.0313x over previous
"""CrossBatchAttention kernel for 8 Trainium2 NeuronCores.

Strategy: tensor-parallel over heads. 16 heads / 8 cores = 2 heads per core.
Each core computes:
  - v     = (hidden @ Wv_slice.T) in [b, e] layout, kt-outer in two waves of
    8 bank-aligned PSUM accumulators so it overlaps the streaming-in of
    hidden.T
  - qT/kT = (W_slice @ hidden.T)  in [e, b] layout (e = local head*128 + d)
  - per (head, q-half) sub-phase: scoresT[k, q] = kT-tiles @ qT (PE-native),
    exp on ACT, diagonal zeroed multiplicatively, row-sums via ones-matmul
    partition reduction, 1/sum broadcast back to [128, q] via a second
    ones-matmul, ctxT[d, q] = v-tiles @ expT accumulated in PSUM, normalized
    on eviction. Epilogue matmuls are software-pipelined into the next
    sub-phase, and the first output-projection groups are interleaved into
    the last sub-phase, so the PE never stalls on the ACT/DVE chain.
  - cross_partial[b, o] = ctxT-tiles @ WoT_slice (accumulate 2 head tiles)
Host: sums the 8 partial cross projections, out = hidden + sigmoid(s)*cross.

All matmul inputs bf16 (fp32 PSUM accumulation). The residual path keeps
hidden in fp32 exactly, and cross contributes only ~2% of output magnitude,
so bf16 compute error is negligible end-to-end.
"""

import numpy as np
import ml_dtypes

B = 2048
H = 2048
NH = 16
HD = 128
NCORES = 8
HL = NH // NCORES          # heads per core = 2
E = HL * HD                # local projection width = 256
P = 128
KT = H // P                # 16 contraction tiles over hidden dim
BT = B // P                # 16 row tiles

_BF16 = ml_dtypes.bfloat16

_compiled = None


def _build():
    import concourse.bass as bass  # noqa: F401
    import concourse.tile as tile
    from concourse import bacc, mybir

    bf = mybir.dt.bfloat16
    f8 = mybir.dt.float8e4
    f32 = mybir.dt.float32
    Exp = mybir.ActivationFunctionType.Exp
    mult = mybir.AluOpType.mult
    # fp8 range scaling: weights x16 on host, exp() rescales scores back
    expscale = float(1.0 / (256.0 * 128.0 ** 0.5))

    nc = bacc.Bacc(
        "TRN2",
        target_bir_lowering=False,
        debug=False,
        enable_asserts=False,
        num_devices=NCORES,
    )

    # inputs are pre-packed on the host to match SBUF layout exactly, so every
    # DMA row is a large contiguous span (descriptor-efficient)
    hT_d = nc.dram_tensor("hT", [P, KT * B], f8, kind="ExternalInput").ap()
    wqT_d = nc.dram_tensor("wqT", [P, KT * E], f8, kind="ExternalInput").ap()
    wkT_d = nc.dram_tensor("wkT", [P, KT * E], f8, kind="ExternalInput").ap()
    wvT_d = nc.dram_tensor("wvT", [P, KT * E], f8, kind="ExternalInput").ap()
    woT_d = nc.dram_tensor("woT", [P, HL * H], bf, kind="ExternalInput").ap()
    antiI_d = nc.dram_tensor("antiI", [P, P], bf, kind="ExternalInput").ap()
    out_d = nc.dram_tensor("out", [B, H], bf, kind="ExternalOutput").ap()

    with tile.TileContext(nc) as tc:
        with (
            tc.tile_pool(name="const", bufs=1) as constp,
            tc.tile_pool(name="work", bufs=1) as workp,
            tc.tile_pool(name="stream", bufs=8) as streamp,
            tc.tile_pool(name="psA", bufs=2, space="PSUM") as psA,
            tc.tile_pool(name="psB", bufs=2, space="PSUM") as psB,
        ):
            # ---------------- constants ----------------
            antiI = constp.tile([P, P], bf)
            ones_k = constp.tile([P, 1], bf)
            nc.gpsimd.memset(ones_k[:], 1.0)
            ones_m = constp.tile([1, P], f32)
            nc.gpsimd.memset(ones_m[:], 1.0 / 16.0)
            zbias = constp.tile([P, 1], f32)
            nc.gpsimd.memset(zbias[:], 0.0)

            # ------- input DMA, ordered to match consumption order -------
            hT_sb = constp.tile([P, KT * B], f8)
            wq_sb = constp.tile([P, KT * E], f8)
            wk_sb = constp.tile([P, KT * E], f8)
            wv_sb = constp.tile([P, KT * E], f8)
            wo_sb = constp.tile([P, HL * H], bf)

            nc.sync.dma_start(wv_sb[:], wvT_d[:])
            for kt in range(KT):
                for c2 in range(2):
                    nc.sync.dma_start(
                        hT_sb[:, kt * B + c2 * 1024: kt * B + (c2 + 1) * 1024],
                        hT_d[:, kt * B + c2 * 1024: kt * B + (c2 + 1) * 1024],
                    )
            nc.sync.dma_start(wq_sb[:], wqT_d[:])
            nc.sync.dma_start(wk_sb[:], wkT_d[:])
            nc.sync.dma_start(wo_sb[:], woT_d[:])
            nc.sync.dma_start(antiI[:], antiI_d[:])

            qT_sb = workp.tile([P, HL * B], f8)   # [d, b] per head at h*B
            kT_sb = workp.tile([P, HL * B], f8)
            v_sb = workp.tile([P, BT * E], bf)    # [b%128, bt*E + e]
            ctxT_sb = workp.tile([P, HL * B], bf)

            # ------- v projection, kt-outer, 2 waves of 8 bank-aligned accs ----
            for wave in range(2):
                vt0 = psA.tile([P, 1024], f32, tag="A", name=f"vt0_{wave}")
                vt1 = psA.tile([P, 1024], f32, tag="A", name=f"vt1_{wave}")
                vt2 = psB.tile([P, 1024], f32, tag="B", name=f"vt2_{wave}")
                vt3 = psB.tile([P, 1024], f32, tag="B", name=f"vt3_{wave}")
                vtiles = [vt0, vt1, vt2, vt3]

                def vacc(j):
                    # accumulator j -> own PSUM bank: slot j//2, col (j%2)*512
                    return vtiles[j // 2][:, (j % 2) * 512: (j % 2) * 512 + 256]

                for kt in range(KT):
                    for j in range(8):
                        bt = wave * 8 + j
                        nc.tensor.matmul(
                            vacc(j),
                            lhsT=hT_sb[:, kt * B + bt * P: kt * B + (bt + 1) * P],
                            rhs=wv_sb[:, kt * E:(kt + 1) * E],
                            start=(kt == 0),
                            stop=(kt == KT - 1),
                        )
                for j in range(8):
                    bt = wave * 8 + j
                    nc.any.tensor_copy(v_sb[:, bt * E:(bt + 1) * E], vacc(j))

            # ---------------- q/k projections (out = [e, b]) ----------------
            for dst, w_sb in ((qT_sb, wq_sb), (kT_sb, wk_sb)):
                for et in range(HL):
                    for bh in range(B // 1024):
                        psp = psA.tile([P, 1024], f32, tag="A", name=f"psp_{et}_{bh}")
                        for kt in range(KT):
                            for b2 in range(2):
                                bc = bh * 2 + b2
                                nc.tensor.matmul(
                                    psp[:, b2 * 512:(b2 + 1) * 512],
                                    lhsT=w_sb[:, kt * E + et * P: kt * E + (et + 1) * P],
                                    rhs=hT_sb[:, kt * B + bc * 512: kt * B + (bc + 1) * 512],
                                    start=(kt == 0),
                                    stop=(kt == KT - 1),
                                )
                        nc.any.tensor_copy(dst[:, et * B + bh * 1024: et * B + (bh + 1) * 1024], psp[:])

            # ------------- output projection helper (interleavable) -----------
            def emit_cross(bt, oh, idx):
                psx = psA.tile([P, 1024], f32, tag="A", name=f"psx_{bt}_{oh}")
                for et in range(HL):
                    for o2 in range(2):
                        nc.tensor.matmul(
                            psx[:, o2 * 512:(o2 + 1) * 512],
                            lhsT=ctxT_sb[:, et * B + bt * P: et * B + (bt + 1) * P],
                            rhs=wo_sb[:, et * H + oh * 1024 + o2 * 512: et * H + oh * 1024 + (o2 + 1) * 512],
                            start=(et == 0),
                            stop=(et == HL - 1),
                        )
                xo = streamp.tile([P, 1024], bf, tag="xo", name=f"xo_{bt}_{oh}")
                if idx % 2 == 0:
                    nc.vector.tensor_copy(xo[:], psx[:])
                else:
                    nc.scalar.copy(xo[:], psx[:])
                eng = nc.sync if idx % 2 == 0 else nc.gpsimd
                eng.dma_start(
                    out_d[bt * P:(bt + 1) * P, oh * 1024:(oh + 1) * 1024], xo[:]
                )

            # ---------------- attention: 4 (head, q-half) sub-phases ----------
            # epilogue of sub-phase s is software-pipelined into sub-phase s+1
            # (part 1 at kt==4, part 2 at kt==8); the first cross-projection
            # groups are interleaved into the last sub-phase's kt loop.
            pending = [None]

            def epilogue_part1():
                st = pending[0]
                if st is None or st.get("p1"):
                    return
                st["p1"] = True
                h, qh, acc = st["h"], st["qh"], st["acc"]
                st["rrows"] = []
                for q2 in range(2):
                    pssum = psA.tile([1, 512], f32, tag="A", name=f"pssum_{h}_{qh}_{q2}")
                    nc.tensor.matmul(
                        pssum[:], lhsT=ones_k[:], rhs=acc[:, q2 * 512:(q2 + 1) * 512],
                        start=True, stop=True,
                    )
                    rrow = workp.tile([1, 512], f32, tag="rrow", bufs=4,
                                      name=f"rrow_{h}_{qh}_{q2}")
                    # sums are O(2500) and well-conditioned; the ~18-bit
                    # approximation is far more accurate than needed and 5x
                    # faster than the multi-pass exact reciprocal, which
                    # would block the in-order DVE stream for ~3.3us
                    nc.vector.reciprocal_approx_fast(rrow[:], pssum[:])
                    st["rrows"].append(rrow)

            def epilogue_part2():
                st = pending[0]
                if st is None or not st.get("p1") or st.get("p2"):
                    return
                st["p2"] = True
                h, qh, ctxps = st["h"], st["qh"], st["ctxps"]
                for q2 in range(2):
                    qc = qh * 2 + q2
                    psrb = psA.tile([P, 512], f32, tag="A", name=f"psrb_{h}_{qh}_{q2}")
                    nc.tensor.matmul(psrb[:], lhsT=ones_m[:], rhs=st["rrows"][q2][:],
                                     start=True, stop=True)
                    rb = workp.tile([P, 512], f32, tag="rb", bufs=4,
                                    name=f"rb_{h}_{qh}_{q2}")
                    nc.vector.tensor_copy(rb[:], psrb[:])
                    nc.vector.tensor_tensor(
                        ctxT_sb[:, h * B + qc * 512: h * B + (qc + 1) * 512],
                        ctxps[:, q2 * 512:(q2 + 1) * 512],
                        rb[:],
                        op=mult,
                    )
                pending[0] = None

            # cross groups interleaved into the last sub-phase (need only
            # qh=0 columns, i.e. sub-phases 0 and 2)
            early_cross = [(0, 0), (0, 1), (1, 0), (1, 1), (2, 0), (2, 1)]

            for h in range(HL):
                for qh in range(2):
                    last = (h == HL - 1 and qh == 1)
                    ctxps = psB.tile([P, 1024], f32, tag="B", name=f"ctxps_{h}_{qh}")
                    acc = workp.tile([P, 1024], bf, tag="acc", bufs=2, name=f"acc_{h}_{qh}")
                    for kt in range(KT):
                        if kt == 4:
                            epilogue_part1()
                        if kt == 8:
                            epilogue_part2()
                        if last and kt >= 10:
                            bt, oh = early_cross[kt - 10]
                            emit_cross(bt, oh, kt)
                        pss = psA.tile([P, 1024], f32, tag="A", name=f"pss_{h}_{qh}_{kt}")
                        for q2 in range(2):
                            qc = qh * 2 + q2
                            nc.tensor.matmul(
                                pss[:, q2 * 512:(q2 + 1) * 512],
                                lhsT=kT_sb[:, h * B + kt * P: h * B + (kt + 1) * P],
                                rhs=qT_sb[:, h * B + qc * 512: h * B + (qc + 1) * 512],
                                start=True,
                                stop=True,
                            )
                        ex = streamp.tile([P, 1024], bf, tag="exp", name=f"ex_{h}_{qh}_{kt}")
                        nc.scalar.activation(ex[:], pss[:], Exp, bias=zbias[:, 0:1], scale=expscale)
                        if kt // 8 == qh:
                            off = kt * P - qh * 1024
                            nc.vector.tensor_tensor(
                                ex[:, off:off + P], ex[:, off:off + P], antiI[:], op=mult
                            )
                        if kt == 0:
                            nc.vector.tensor_copy(acc[:], ex[:])
                        else:
                            nc.vector.tensor_add(acc[:], acc[:], ex[:])
                        for q2 in range(2):
                            nc.tensor.matmul(
                                ctxps[:, q2 * 512:(q2 + 1) * 512],
                                lhsT=v_sb[:, kt * E + h * P: kt * E + h * P + P],
                                rhs=ex[:, q2 * 512:(q2 + 1) * 512],
                                start=(kt == 0),
                                stop=(kt == KT - 1),
                            )
                    epilogue_part1()
                    epilogue_part2()
                    pending[0] = {"h": h, "qh": qh, "acc": acc, "ctxps": ctxps}

            # ---------------- remaining output projection ----------------
            idx = len(early_cross)
            done = set(early_cross)
            for bt in range(BT):
                if bt == 3:
                    epilogue_part1()
                if bt == 5:
                    epilogue_part2()
                for oh in range(2):
                    if (bt, oh) in done:
                        continue
                    emit_cross(bt, oh, idx)
                    idx += 1

    nc.compile()
    return nc


def _get_compiled():
    global _compiled
    if _compiled is None:
        _compiled = _build()
    return _compiled


def _numpy_reference(hidden_states, attention_mask, Wq, Wk, Wv, Wo, scale_param):
    hs = np.asarray(hidden_states, np.float64)
    q = (hs @ np.asarray(Wq, np.float64).T).reshape(B, NH, HD).transpose(1, 0, 2)
    k = (hs @ np.asarray(Wk, np.float64).T).reshape(B, NH, HD).transpose(1, 0, 2)
    v = (hs @ np.asarray(Wv, np.float64).T).reshape(B, NH, HD).transpose(1, 0, 2)
    scores = np.einsum("hqd,hkd->hqk", q, k) / (HD ** 0.5)
    eye = np.eye(B, dtype=bool)
    scores = np.where(eye[None, :, :], -np.inf, scores)
    mask = np.asarray(attention_mask, bool)
    scores = np.where(mask[None, None, :], scores, -np.inf)
    m = scores.max(axis=-1, keepdims=True)
    m = np.where(np.isfinite(m), m, 0.0)
    e = np.exp(scores - m)
    s = e.sum(axis=-1, keepdims=True)
    attn = np.where(s > 0, e / np.maximum(s, 1e-300), 0.0)
    ctx = np.einsum("hqk,hkd->hqd", attn, v)
    ctx = ctx.transpose(1, 0, 2).reshape(B, H)
    cross = ctx @ np.asarray(Wo, np.float64).T
    scale = 1.0 / (1.0 + np.exp(-float(np.asarray(scale_param).reshape(-1)[0])))
    return (hs + scale * cross).astype(np.float32)


LAST_RESULTS = None


def kernel(hidden_states, attention_mask, Wq, Wk, Wv, Wo, scale_param):
    hs = np.asarray(hidden_states, np.float32)
    mask = np.asarray(attention_mask, bool)
    if not mask.all():
        return _numpy_reference(hidden_states, mask, Wq, Wk, Wv, Wo, scale_param)

    from concourse import bass_utils

    nc = _get_compiled()

    def pack(a):
        # [T*128, W] -> [128, T*W]: row p holds tile-t's row p for every t,
        # matching the SBUF destination layout exactly (contiguous DMA rows)
        t = a.shape[0] // P
        return np.ascontiguousarray(
            a.reshape(t, P, a.shape[1]).transpose(1, 0, 2).reshape(P, t * a.shape[1])
        )

    from concourse import mybir as _mybir
    _F8 = _mybir.dt.np(_mybir.dt.float8e4)

    hT = pack(hs.T.astype(_F8))
    antiI = (1.0 - np.eye(P, dtype=np.float32)).astype(_BF16)
    Wq = np.asarray(Wq, np.float32)
    Wk = np.asarray(Wk, np.float32)
    Wv = np.asarray(Wv, np.float32)
    Wo = np.asarray(Wo, np.float32)

    in_maps = []
    for c in range(NCORES):
        rs = slice(c * E, (c + 1) * E)
        in_maps.append({
            "hT": hT,
            "wqT": pack((Wq[rs, :].T * np.float32(16.0)).astype(_F8)),
            "wkT": pack((Wk[rs, :].T * np.float32(16.0)).astype(_F8)),
            "wvT": pack((Wv[rs, :].T * np.float32(16.0)).astype(_F8)),
            "woT": pack(Wo[:, rs].T.astype(_BF16)),
            "antiI": antiI,
        })

    import os
    res = bass_utils.run_bass_kernel_spmd(
        nc, in_maps, core_ids=list(range(NCORES)),
        trace=bool(os.environ.get("KERNEL_TRACE")),
    )
    global LAST_RESULTS
    LAST_RESULTS = res

    cross = np.zeros((B, H), np.float32)
    for r in res.results:
        cross += np.asarray(r["out"], np.float32)
    scale = np.float32(1.0 / (1.0 + np.exp(-float(np.asarray(scale_param).reshape(-1)[0]))))
    return (hs + scale * cross).astype(np.float32)
